# revision 17
# baseline (speedup 1.0000x reference)
"""Causal self-attention with RoPE on 8 Trainium2 NeuronCores.

Sharding: Megatron-style head parallelism. 16 heads / 8 cores = 2 heads per
core. Each core computes q/k/v projections for its 2 heads (column-parallel),
full causal attention for those heads, and a partial output projection
(row-parallel slice of w_o). The host sums the 8 partial outputs.

v3 design (vs the v2 baseline at ~982us):
- x and all weights travel/store as bf16 (halves DMA + SBUF); all on-chip
  attention math stays f32/f32r (fp32r runs at full PE rate for moving
  dim >= 256, so bf16 buys nothing on the PE).
- The causal mask costs no PE time: exp is computed on the raw logits
  (|logit| < ~8 so exp never overflows) and the diagonal triangle is zeroed
  in-place by gpsimd.affine_select (idle Pool engine, constant pattern).
- Softmax denominators cost ~no PE time: the Pool engine accumulates
  R += pex per kv tile; one ones-matmul per q-group reduces R over
  partitions (vs one per kv tile before).
- Diagonal score/exp/PV tiles are narrowed to the valid column range
  (512/384/256/256 for dg=0..3) - ~25% less attention area.
- Projection t-tile TT=512 (wider, fewer matmuls).
- Next batch's x tiles are prefetched at the start of the current batch's
  attention phase so the proj matmuls never wait on DMA behind y writes.
- Normalization/output-projection is emitted in two deferred stages
  (stage1: sums/reciprocal/normalize, stage2: output projection) threaded
  between the next group's tiles so the PE never waits on the DVE chain.
"""

import numpy as np

B, T, D = 4, 2048, 2048
H, DH = 16, 128
NCORES = 8
HPC = H // NCORES  # heads per core
THETA = 10000.0

TT = 512  # projection t-tile (moving dim of q/k projection matmuls)
TQ = 512  # attention q-group width
TK = 128  # kv tile (contraction chunk of PV / partition dim of ST)


def _rope_tables(seq_len, d_head, theta):
    # Matches reference.rope_cos_sin numerics, then transposes to [dh, t]
    # and folds the rotate-half sign into sin.
    inv_freq = 1.0 / (theta ** (np.arange(0, d_head, 2, dtype=np.float32) / d_head))
    t = np.arange(seq_len, dtype=np.float32)
    freqs = np.einsum("i,j->ij", t, inv_freq)
    emb = np.concatenate([freqs, freqs], axis=-1)  # [T, dh]
    cosT = np.ascontiguousarray(np.cos(emb).astype(np.float32).T)  # [dh, T]
    sinT = np.ascontiguousarray(np.sin(emb).astype(np.float32).T)
    sgn = np.ones((d_head, 1), np.float32)
    sgn[: d_head // 2] = -1.0
    return cosT, sinT * sgn


def _legalize_waits(nc, mybir):
    """Walrus on this toolchain refuses more than one embedded sync wait
    per engine instruction. Hoist extra waits into standalone
    EventSemaphore instructions on the same engine queue (the sequencer
    executes them in-stream before the instruction, same gating)."""
    n = 0
    for f in nc.m.functions:
        for bb in f.blocks:
            out = []
            for inst in bb.instructions:
                si = inst.sync_info
                if (si and si.on_wait and len(si.on_wait) > 1
                        and not isinstance(inst, mybir.InstEventSemaphore)):
                    for w in si.on_wait[:-1]:
                        out.append(mybir.InstEventSemaphore(
                            name=f"WH-{n}", engine=inst.engine,
                            sync_info=mybir.SyncInfo(
                                on_wait=[w], on_update=[])))
                        n += 1
                    inst.sync_info = mybir.SyncInfo(
                        on_wait=[si.on_wait[-1]],
                        on_update=list(si.on_update))
                out.append(inst)
            bb.instructions = out
    return n


def _build_nc(b_sz, t_sz, d_sz, legalize=True):
    import concourse.bass as bass
    import concourse.tile as tile
    from concourse import mybir

    f32 = mybir.dt.float32
    f32r = mybir.dt.float32r
    bf16 = mybir.dt.bfloat16
    EXP = mybir.ActivationFunctionType.Exp

    DC = d_sz // 128         # contraction chunks
    NQG = t_sz // TQ         # q groups per (batch, head)
    NKT = t_sz // TK         # kv tiles
    KPG = TQ // TK           # kv tiles per q group (diagonal span)

    nc = bass.Bass("TRN2", target_bir_lowering=False, debug=False,
                   enable_asserts=False, dynamic_dma_scratch_size=2048)

    xT = nc.dram_tensor("xT", [b_sz, d_sz, t_sz], bf16, kind="ExternalInput")
    wq = nc.dram_tensor("wq", [d_sz, HPC * DH], bf16, kind="ExternalInput")
    wk = nc.dram_tensor("wk", [d_sz, HPC * DH], bf16, kind="ExternalInput")
    wv = nc.dram_tensor("wv", [d_sz, HPC * DH], bf16, kind="ExternalInput")
    wo = nc.dram_tensor("wo", [HPC * DH, d_sz], bf16, kind="ExternalInput")
    cos = nc.dram_tensor("cos", [DH, t_sz], f32, kind="ExternalInput")
    sin = nc.dram_tensor("sin", [DH, t_sz], f32, kind="ExternalInput")
    one = nc.dram_tensor("one", [128, 128], f32, kind="ExternalInput")
    y = nc.dram_tensor("y", [b_sz, t_sz, d_sz], f32, kind="ExternalOutput")

    xT_r = xT.ap().rearrange("b (dc p) t -> b p dc t", p=128)
    wq_r = wq.ap().rearrange("(dc p) n -> p dc n", p=128)
    wk_r = wk.ap().rearrange("(dc p) n -> p dc n", p=128)
    wv_r = wv.ap().rearrange("(dc p) n -> p dc n", p=128)
    wo_r = wo.ap().rearrange("(h p) n -> p h n", p=128)
    y_r = y.ap()

    # diagonal tile narrowing: valid col offset per dg (last one kept at
    # 256 wide so the fp32r moving dim stays >= 256)
    dg_off = {0: 0, 1: 128, 2: 256, 3: 256}

    with tile.TileContext(nc) as tc:
        with (
            tc.tile_pool(name="consts", bufs=1) as consts,
            tc.tile_pool(name="wpool", bufs=1) as wpool,
            tc.tile_pool(name="qkv", bufs=1) as qkv,
            tc.tile_pool(name="xpool", bufs=4) as xpool,
            tc.tile_pool(name="rope", bufs=2) as rope,
            tc.tile_pool(name="pex", bufs=3) as pexp,
            tc.tile_pool(name="sax", bufs=1) as sax,
            tc.tile_pool(name="otn", bufs=6) as otnp,
            tc.tile_pool(name="psS", bufs=2, space="PSUM") as psS,
            tc.tile_pool(name="psO", bufs=2, space="PSUM") as psO,
            tc.tile_pool(name="psR", bufs=1, space="PSUM") as psR,
            tc.tile_pool(name="psY", bufs=2, space="PSUM") as psY,
        ):
            cos_sb = consts.tile([DH, t_sz], f32)
            sin_sb = consts.tile([DH, t_sz], f32)
            ones_sb = consts.tile([128, 1], f32r)
            onesrow_sb = consts.tile([1, 128], f32r)

            wq_sb = wpool.tile([128, DC, HPC * DH], bf16)
            wk_sb = wpool.tile([128, DC, HPC * DH], bf16)
            wv_sb = wpool.tile([128, DC, HPC * DH], bf16)
            wo_sb = wpool.tile([128, HPC, d_sz], bf16)

            # first-needed data first: the first x tile and q/k/v weight
            # chunks feed the very first matmuls, so their DMAs go at the
            # head of every queue
            xt_first = xpool.tile([128, DC, TT], bf16, tag="xt",
                                  name="xt_first")
            for dc in range(DC):
                nc.sync.dma_start(xt_first[:, dc, :],
                                  xT_r[0, :, dc, 0:TT])
                nc.sync.dma_start(wq_sb[:, dc, :], wq_r[:, dc, :])
                nc.sync.dma_start(wk_sb[:, dc, :], wk_r[:, dc, :])
                nc.sync.dma_start(wv_sb[:, dc, :], wv_r[:, dc, :])
            # cos/sin feed the very first RoPE (~10us in) - ahead of the
            # remaining x tiles so they don't queue behind 6MB of x
            for i in range(t_sz // TT):
                sl = slice(i * TT, (i + 1) * TT)
                nc.sync.dma_start(cos_sb[:, sl], cos.ap()[:, sl])
                nc.sync.dma_start(sin_sb[:, sl], sin.ap()[:, sl])

            def load_consts():
                # nothing here is needed before the first attention group
                nc.sync.dma_start(ones_sb[:], one.ap()[:, 0:1].bitcast(f32r))
                nc.sync.dma_start(onesrow_sb[:],
                                  one.ap()[0:1, :].bitcast(f32r))
                for hh in range(HPC):
                    for nch in range(d_sz // 512):
                        nsl = slice(nch * 512, (nch + 1) * 512)
                        nc.sync.dma_start(wo_sb[:, hh, nsl],
                                          wo_r[:, hh, nsl])

            # deferred normalize/output-projection closures, run one group late
            pending = []

            def pop(queue):
                if queue:
                    queue.pop(0)()

            # x tiles for a batch are created (and their DMAs emitted) ahead
            # of time so the loads overlap the previous batch's attention
            xt_tiles = {0: None}

            def prefetch_x(b):
                tiles = []
                for tt in range(t_sz // TT):
                    if b == 0 and tt == 0:
                        tiles.append(xt_first)
                        continue
                    xt = xpool.tile([128, DC, TT], bf16, tag="xt", name="xt")
                    tsl = slice(tt * TT, (tt + 1) * TT)
                    for dc in range(DC):
                        nc.sync.dma_start(xt[:, dc, :], xT_r[b, :, dc, tsl])
                    tiles.append(xt)
                return tiles

            xts = prefetch_x(0)
            for b in range(b_sz):
                # ---------------- phase A: projections + RoPE ----------
                qT = [qkv.tile([DH, t_sz], f32r, tag=f"qT{h}", name=f"qT{h}")
                      for h in range(HPC)]
                kT = [qkv.tile([DH, t_sz], f32r, tag=f"kT{h}", name=f"kT{h}")
                      for h in range(HPC)]
                vv = qkv.tile([128, NKT, HPC * DH], f32r, tag="v", name="v")

                for tt in range(t_sz // TT):
                    tsl = slice(tt * TT, (tt + 1) * TT)
                    xt = xts[tt]
                    if b == 0 and tt == 0:
                        load_consts()

                    for h in range(HPC):
                        hs = slice(h * DH, (h + 1) * DH)
                        for dst, w_sb in ((qT[h], wq_sb), (kT[h], wk_sb)):
                            pp = psS.tile([DH, TT], f32, tag="st")
                            for dc in range(DC):
                                nc.tensor.matmul(
                                    pp[:],
                                    w_sb[:, dc, hs],
                                    xt[:, dc, :],
                                    start=(dc == 0), stop=(dc == DC - 1),
                                )
                            # RoPE: dst = pp*cos + swap(pp)*sin_signed
                            sh = rope.tile([DH, TT], f32, tag="sh")
                            nc.vector.tensor_mul(
                                sh[0:64, :], pp[64:128, :], sin_sb[0:64, tsl])
                            nc.vector.tensor_mul(
                                sh[64:128, :], pp[0:64, :],
                                sin_sb[64:128, tsl])
                            nc.vector.tensor_mul(dst[:, tsl], pp[:],
                                                 cos_sb[:, tsl])
                            nc.vector.tensor_add(dst[:, tsl], dst[:, tsl],
                                                 sh[:])
                        # drain deferred attention work from the previous
                        # batch between projection chains (keeps PE dense)
                        if tt == 0:
                            pop(pending)

                    for ts2 in range(TT // TK):
                        vp = psS.tile([TK, HPC * DH], f32, tag="st")
                        for dc in range(DC):
                            nc.tensor.matmul(
                                vp[:],
                                xt[:, dc, ts2 * TK:(ts2 + 1) * TK],
                                wv_sb[:, dc, :],
                                start=(dc == 0), stop=(dc == DC - 1),
                            )
                        kv_i = tt * (TT // TK) + ts2
                        nc.scalar.copy(vv[:, kv_i, :], vp[:])

                # prefetch next batch's x now: the DMAs enter the queues
                # before this batch's y writes
                if b + 1 < b_sz:
                    xts = prefetch_x(b + 1)

                # ---------------- phase B + C: attention + out proj ----
                otn_tiles = {}
                for h in range(HPC):
                    for qi in range(NQG):
                        nkv = KPG * (qi + 1)
                        outp = psO.tile([DH, TQ], f32, tag="outT")
                        sump = psR.tile([1, TQ], f32, tag="sums",
                                        name="sump")
                        for ki in range(nkv):
                            dg = ki - KPG * qi
                            off = dg_off[dg] if dg >= 0 else 0
                            qsl = slice(qi * TQ + off, (qi + 1) * TQ)
                            stp = psS.tile([TK, TQ], f32, tag="st")
                            nc.tensor.matmul(
                                stp[:, off:],
                                kT[h][:, ki * TK:(ki + 1) * TK],
                                qT[h][:, qsl],
                                start=True, stop=True,
                            )
                            pex = pexp.tile([TK, TQ], f32r, tag="pex")
                            nc.scalar.activation(pex[:, off:], stp[:, off:],
                                                 EXP)
                            if dg >= 0:
                                # zero the invalid triangle of exp in place
                                # on the Pool engine: keep iff
                                # col - row - (valid_start - off) >= 0
                                base = off - (dg * TK)  # 0 or -128 (dg=3)
                                blk = 2 * TK if dg == 3 else TK
                                nc.gpsimd.affine_select(
                                    out=pex[:, off:off + blk],
                                    in_=pex[:, off:off + blk],
                                    compare_op=mybir.AluOpType.is_ge,
                                    fill=0.0,
                                    base=base,
                                    pattern=[[1, blk]],
                                    channel_multiplier=-1,
                                )
                            nc.tensor.matmul(
                                outp[:, off:],
                                vv[:, ki, h * DH:(h + 1) * DH],
                                pex[:, off:],
                                start=(ki == 0), stop=(ki == nkv - 1),
                                skip_group_check=True,
                            )
                            nc.tensor.matmul(
                                sump[:, off:],
                                ones_sb[:],
                                pex[:, off:],
                                start=(ki == 0), stop=(ki == nkv - 1),
                                skip_group_check=True,
                            )

                        def norm(h=h, qi=qi, outp=outp, sump=sump, b=b,
                                 ot=otn_tiles):
                            # one q-group late: PE never waits on the DVE
                            # reciprocal chain
                            ssb = sax.tile([1, TQ], f32r, tag="ssb", bufs=2,
                                           name="ssb")
                            nc.scalar.copy(ssb[:], sump[:])
                            rbc = psR.tile([DH, TQ], f32, tag="bc",
                                           name="rbc")
                            nc.tensor.matmul(rbc[:], onesrow_sb[:], ssb[:],
                                             start=True, stop=True)
                            rcp = sax.tile([DH, TQ], f32, tag="rcp", bufs=2,
                                           name="rcp")
                            nc.vector.reciprocal(rcp[:], rbc[:])
                            otn = otnp.tile([DH, TQ], bf16, tag="otn",
                                            name="otn")
                            nc.vector.tensor_mul(otn[:], outp[:], rcp[:])
                            ot[(h, qi)] = otn
                            if h != HPC - 1:
                                return
                            for tc2 in range(TQ // TK):
                                tq0 = qi * TQ + tc2 * TK
                                for nch in range(d_sz // 512):
                                    yp = psY.tile([TK, 512], f32,
                                                  tag="y", name="yp")
                                    for hh in range(HPC):
                                        nc.tensor.matmul(
                                            yp[:],
                                            ot[(hh, qi)][
                                                :, tc2 * TK:
                                                (tc2 + 1) * TK],
                                            wo_sb[:, hh,
                                                  nch * 512:
                                                  (nch + 1) * 512],
                                            start=(hh == 0),
                                            stop=(hh == HPC - 1),
                                        )
                                    ysb = pexp.tile([TK, 512], f32,
                                                    tag="ysb", bufs=3,
                                                    name="ysb")
                                    if nch % 2 == 0:
                                        nc.scalar.copy(ysb[:], yp[:])
                                    else:
                                        nc.vector.tensor_copy(ysb[:],
                                                              yp[:])
                                    nc.sync.dma_start(
                                        y_r[b, tq0:tq0 + TK,
                                            nch * 512:(nch + 1) * 512],
                                        ysb[:])

                        pending.append(norm)
                        if len(pending) > 1:
                            pending.pop(0)()
            for fn in pending:
                fn()
    if legalize:
        _legalize_waits(nc, mybir)
    return nc


_NC_CACHE = {}
LAST_RESULT = None


def _get_nc(b_sz, t_sz, d_sz):
    key = (b_sz, t_sz, d_sz)
    if key not in _NC_CACHE:
        _NC_CACHE[key] = _build_nc(b_sz, t_sz, d_sz)
    return _NC_CACHE[key]


def kernel(x, w_q, w_k, w_v, w_o):
    import ml_dtypes
    from concourse.bass_utils import run_bass_kernel_spmd

    bf = ml_dtypes.bfloat16
    b_sz, t_sz, d_sz = x.shape
    scale = np.float32(1.0 / np.sqrt(DH))

    xT = np.ascontiguousarray(
        np.asarray(x, np.float32).transpose(0, 2, 1)).astype(bf)
    w_q = np.asarray(w_q, np.float32)
    w_k = np.asarray(w_k, np.float32)
    w_v = np.asarray(w_v, np.float32)
    w_o = np.asarray(w_o, np.float32)
    cosT, sinT = _rope_tables(t_sz, DH, THETA)

    in_maps = []
    for c in range(NCORES):
        cs = slice(c * HPC * DH, (c + 1) * HPC * DH)
        in_maps.append({
            "xT": xT,
            "wq": np.ascontiguousarray(w_q[:, cs] * scale).astype(bf),
            "wk": np.ascontiguousarray(w_k[:, cs]).astype(bf),
            "wv": np.ascontiguousarray(w_v[:, cs]).astype(bf),
            "wo": np.ascontiguousarray(w_o[cs, :]).astype(bf),
            "cos": cosT,
            "sin": sinT,
            "one": np.ones((128, 128), np.float32),
        })

    nc = _get_nc(b_sz, t_sz, d_sz)
    res = run_bass_kernel_spmd(nc, in_maps, core_ids=list(range(NCORES)))
    global LAST_RESULT
    LAST_RESULT = res

    out = res.results[0]["y"].astype(np.float32, copy=True)
    for c in range(1, NCORES):
        out += res.results[c]["y"]
    return out


# revision 23
# speedup vs baseline: 1.1945x; 1.1945x over previous
"""Causal self-attention with RoPE on 8 Trainium2 NeuronCores.

Sharding: Megatron-style head parallelism. 16 heads / 8 cores = 2 heads per
core. Each core computes q/k/v projections for its 2 heads (column-parallel),
full causal attention for those heads, and a partial output projection
(row-parallel slice of w_o). The host sums the 8 partial outputs.

v5 design (vs the 982us v2 baseline). Measured facts this build exploits:
- bf16 matmuls stream 2 rows/cycle (116ns for a 512-wide moving dim) vs
  fp32r's 1 row/cycle, BUT back-to-back matmuls accumulating into the SAME
  PSUM bank stall ~143ns between instructions, while matmuls targeting
  alternating banks issue ~15ns apart. So every tensor is bf16 on the PE
  and every long accumulation chain is split across two PSUM banks (even
  dc -> bank A, odd dc -> bank B) and merged during RoPE / the v copy.
- The causal mask costs no PE time: exp runs on raw logits (|logit| < ~8)
  and gpsimd.affine_select zeroes the invalid triangle in place.
- Diagonal tiles are narrowed to the valid column range (saves exp area).
- The attention inner loop is software-pipelined one tile deep (score_ki
  is issued before PV_{ki-1}) and output-projection y-tiles from the
  previous q group are threaded two-per-iteration into the stream, so the
  PE has queued work while ACT computes exp.
- Softmax denominators accumulate in PSUM via per-tile ones-matmuls (also
  narrowed); normalization is deferred one group (reciprocal chain never
  blocks the PE).
- x travels as bf16 (halves DMA); cos/sin DMAs are emitted before the
  later x tiles so the first RoPE is never queued behind 6MB of x.
"""

import numpy as np

B, T, D = 4, 2048, 2048
H, DH = 16, 128
NCORES = 8
HPC = H // NCORES  # heads per core
THETA = 10000.0

TT = 512  # projection t-tile (moving dim of q/k projection matmuls)
TQ = 512  # attention q-group width
TK = 128  # kv tile (contraction chunk of PV / partition dim of ST)


def _rope_tables(seq_len, d_head, theta):
    inv_freq = 1.0 / (theta ** (np.arange(0, d_head, 2, dtype=np.float32) / d_head))
    t = np.arange(seq_len, dtype=np.float32)
    freqs = np.einsum("i,j->ij", t, inv_freq)
    emb = np.concatenate([freqs, freqs], axis=-1)  # [T, dh]
    cosT = np.ascontiguousarray(np.cos(emb).astype(np.float32).T)  # [dh, T]
    sinT = np.ascontiguousarray(np.sin(emb).astype(np.float32).T)
    sgn = np.ones((d_head, 1), np.float32)
    sgn[: d_head // 2] = -1.0
    # halves swapped: the kernel's rotate-half multiplies then read both
    # SBUF operands at the same base partition (a walrus requirement)
    return cosT, np.roll(sinT * sgn, d_head // 2, axis=0)


def _legalize_waits(nc, mybir):
    """Walrus on this toolchain refuses more than one embedded sync wait
    per engine instruction. Hoist extra waits into standalone
    EventSemaphore instructions on the same engine queue."""
    n = 0
    for f in nc.m.functions:
        for bb in f.blocks:
            out = []
            for inst in bb.instructions:
                si = inst.sync_info
                if (si and si.on_wait and len(si.on_wait) > 1
                        and not isinstance(inst, mybir.InstEventSemaphore)):
                    for w in si.on_wait[:-1]:
                        out.append(mybir.InstEventSemaphore(
                            name=f"WH-{n}", engine=inst.engine,
                            sync_info=mybir.SyncInfo(
                                on_wait=[w], on_update=[])))
                        n += 1
                    inst.sync_info = mybir.SyncInfo(
                        on_wait=[si.on_wait[-1]],
                        on_update=list(si.on_update))
                out.append(inst)
            bb.instructions = out
    return n


def _build_nc(b_sz, t_sz, d_sz, legalize=True):
    import concourse.bass as bass
    import concourse.tile as tile
    from concourse import mybir

    f32 = mybir.dt.float32
    f32r = mybir.dt.float32r
    bf16 = mybir.dt.bfloat16
    EXP = mybir.ActivationFunctionType.Exp

    DC = d_sz // 128         # contraction chunks
    NQG = t_sz // TQ         # q groups per (batch, head)
    NKT = t_sz // TK         # kv tiles
    KPG = TQ // TK           # kv tiles per q group (diagonal span)

    nc = bass.Bass("TRN2", target_bir_lowering=False, debug=False,
                   enable_asserts=False, dynamic_dma_scratch_size=2048)

    xT = nc.dram_tensor("xT", [b_sz, d_sz, t_sz], bf16, kind="ExternalInput")
    wq = nc.dram_tensor("wq", [d_sz, HPC * DH], bf16, kind="ExternalInput")
    wk = nc.dram_tensor("wk", [d_sz, HPC * DH], bf16, kind="ExternalInput")
    wv = nc.dram_tensor("wv", [d_sz, HPC * DH], bf16, kind="ExternalInput")
    wo = nc.dram_tensor("wo", [HPC * DH, d_sz], bf16, kind="ExternalInput")
    cos = nc.dram_tensor("cos", [DH, t_sz], f32, kind="ExternalInput")
    sin = nc.dram_tensor("sin", [DH, t_sz], f32, kind="ExternalInput")
    one = nc.dram_tensor("one", [128, 128], f32, kind="ExternalInput")
    oneb = nc.dram_tensor("oneb", [128, 128], bf16, kind="ExternalInput")
    y = nc.dram_tensor("y", [b_sz, t_sz, d_sz], f32, kind="ExternalOutput")

    xT_r = xT.ap().rearrange("b (dc p) t -> b p dc t", p=128)
    wq_r = wq.ap().rearrange("(dc p) n -> p dc n", p=128)
    wk_r = wk.ap().rearrange("(dc p) n -> p dc n", p=128)
    wv_r = wv.ap().rearrange("(dc p) n -> p dc n", p=128)
    wo_r = wo.ap().rearrange("(h p) n -> p h n", p=128)
    y_r = y.ap()

    # diagonal tile narrowing: valid col offset per dg
    dg_off = {0: 0, 1: 128, 2: 256, 3: 256}

    with tile.TileContext(nc) as tc:
        with (
            tc.tile_pool(name="consts", bufs=1) as consts,
            tc.tile_pool(name="wpool", bufs=1) as wpool,
            tc.tile_pool(name="qkv", bufs=1) as qkv,
            tc.tile_pool(name="xpool", bufs=4) as xpool,
            tc.tile_pool(name="rope", bufs=2) as rope,
            tc.tile_pool(name="pex", bufs=6) as pexp,
            tc.tile_pool(name="sax", bufs=1) as sax,
            tc.tile_pool(name="otn", bufs=6) as otnp,
            tc.tile_pool(name="psS", bufs=2, space="PSUM") as psS,
            tc.tile_pool(name="psO", bufs=2, space="PSUM") as psO,
            tc.tile_pool(name="psR", bufs=1, space="PSUM") as psR,
            tc.tile_pool(name="psY", bufs=2, space="PSUM") as psY,
        ):
            cos_sb = consts.tile([DH, t_sz], f32)
            sin_sb = consts.tile([DH, t_sz], f32)
            onesb_sb = consts.tile([128, 1], bf16)
            onesrow_sb = consts.tile([1, 128], f32r)

            wq_sb = wpool.tile([128, DC, HPC * DH], bf16)
            wk_sb = wpool.tile([128, DC, HPC * DH], bf16)
            wv_sb = wpool.tile([128, DC, HPC * DH], bf16)
            wo_sb = wpool.tile([128, HPC, d_sz], bf16)

            xt_first = xpool.tile([128, DC, TT], bf16, tag="xt",
                                  name="xt_first")
            for dc in range(DC):
                nc.sync.dma_start(xt_first[:, dc, :],
                                  xT_r[0, :, dc, 0:TT])
                nc.sync.dma_start(wq_sb[:, dc, :], wq_r[:, dc, :])
                nc.sync.dma_start(wk_sb[:, dc, :], wk_r[:, dc, :])
                nc.sync.dma_start(wv_sb[:, dc, :], wv_r[:, dc, :])
            # cos/sin feed the very first RoPE (~10us in) - ahead of the
            # remaining x tiles so they don't queue behind 6MB of x
            for i in range(t_sz // TT):
                sl = slice(i * TT, (i + 1) * TT)
                nc.sync.dma_start(cos_sb[:, sl], cos.ap()[:, sl])
                nc.sync.dma_start(sin_sb[:, sl], sin.ap()[:, sl])

            def load_consts():
                nc.sync.dma_start(onesb_sb[:], oneb.ap()[:, 0:1])
                nc.sync.dma_start(onesrow_sb[:],
                                  one.ap()[0:1, :].bitcast(f32r))
                for hh in range(HPC):
                    for nch in range(d_sz // 512):
                        nsl = slice(nch * 512, (nch + 1) * 512)
                        nc.sync.dma_start(wo_sb[:, hh, nsl],
                                          wo_r[:, hh, nsl])

            # deferred work: norm closures (one group late) and fine-grained
            # output-projection y-tile closures threaded into later streams
            pending_norm = []
            pending_y = []

            def pop_norm():
                if pending_norm:
                    pending_norm.pop(0)()

            def pop_y(k=2):
                for _ in range(k):
                    if pending_y:
                        pending_y.pop(0)()

            def prefetch_x(b):
                tiles = []
                for tt in range(t_sz // TT):
                    if b == 0 and tt == 0:
                        tiles.append(xt_first)
                        continue
                    xt = xpool.tile([128, DC, TT], bf16, tag="xt", name="xt")
                    tsl = slice(tt * TT, (tt + 1) * TT)
                    for dc in range(DC):
                        nc.sync.dma_start(xt[:, dc, :], xT_r[b, :, dc, tsl])
                    tiles.append(xt)
                return tiles

            xts = prefetch_x(0)
            for b in range(b_sz):
                # ---------------- phase A: projections + RoPE ----------
                pop_norm()
                qT = [qkv.tile([DH, t_sz], bf16, tag=f"qT{h}", name=f"qT{h}")
                      for h in range(HPC)]
                kT = [qkv.tile([DH, t_sz], bf16, tag=f"kT{h}", name=f"kT{h}")
                      for h in range(HPC)]
                vv = qkv.tile([128, NKT, HPC * DH], bf16, tag="v", name="v")

                for tt in range(t_sz // TT):
                    tsl = slice(tt * TT, (tt + 1) * TT)
                    xt = xts[tt]
                    if b == 0 and tt == 0:
                        load_consts()

                    for h in range(HPC):
                        hs = slice(h * DH, (h + 1) * DH)
                        for dst, w_sb in ((qT[h], wq_sb), (kT[h], wk_sb)):
                            # split the 16-chunk contraction across two
                            # PSUM banks so the PE never stalls on the
                            # same-bank accumulate commit
                            ppA = psS.tile([DH, TT], f32, tag="st")
                            ppB = psO.tile([DH, TT], f32, tag="outT")
                            for dc in range(DC):
                                nc.tensor.matmul(
                                    ppA if dc % 2 == 0 else ppB,
                                    w_sb[:, dc, hs],
                                    xt[:, dc, :],
                                    start=(dc < 2), stop=(dc >= DC - 2),
                                )
                            pop_y(2)
                            # merge halves + RoPE on DVE (ppB staged
                            # through SBUF by ACT to keep DVE ops at one
                            # PSUM operand each)
                            pbs = rope.tile([DH, TT], f32, tag="pbs")
                            nc.scalar.copy(pbs[:], ppB[:])
                            pm = rope.tile([DH, TT], f32, tag="pm")
                            nc.vector.tensor_add(pm[:], ppA[:], pbs[:])
                            # sin table arrives with halves pre-swapped so
                            # both SBUF inputs share a base partition
                            sh = rope.tile([DH, TT], bf16, tag="sh")
                            nc.vector.tensor_mul(
                                sh[0:64, :], pm[64:128, :],
                                sin_sb[64:128, tsl])
                            nc.vector.tensor_mul(
                                sh[64:128, :], pm[0:64, :],
                                sin_sb[0:64, tsl])
                            t2 = rope.tile([DH, TT], bf16, tag="t2")
                            nc.vector.tensor_mul(t2[:], pm[:], cos_sb[:, tsl])
                            nc.vector.tensor_add(dst[:, tsl], t2[:], sh[:])

                    for ts2 in range(TT // TK):
                        vpA = psS.tile([TK, HPC * DH], f32, tag="st")
                        vpB = psO.tile([TK, HPC * DH], f32, tag="outT")
                        for dc in range(DC):
                            nc.tensor.matmul(
                                vpA if dc % 2 == 0 else vpB,
                                xt[:, dc, ts2 * TK:(ts2 + 1) * TK],
                                wv_sb[:, dc, :],
                                start=(dc < 2), stop=(dc >= DC - 2),
                            )
                        pop_y(2)
                        kv_i = tt * (TT // TK) + ts2
                        vbs = rope.tile([TK, HPC * DH], f32, tag="vbs")
                        nc.scalar.copy(vbs[:], vpB[:])
                        nc.vector.tensor_add(vv[:, kv_i, :], vpA[:], vbs[:])

                # prefetch next batch's x now: the DMAs enter the queues
                # before this batch's y writes
                if b + 1 < b_sz:
                    xts = prefetch_x(b + 1)

                # ---------------- phase B: attention (+ threaded o-proj) --
                otn_tiles = {}
                for h in range(HPC):
                    for qi in range(NQG):
                        nkv = KPG * (qi + 1)
                        outp = psO.tile([DH, TQ], f32, tag="outT")
                        sump = psR.tile([1, TQ], f32, tag="sums",
                                        name="sump")
                        tiles = []  # (ki, off, pex)
                        for ki in range(nkv):
                            dg = ki - KPG * qi
                            off = dg_off[dg] if dg >= 0 else 0
                            qsl = slice(qi * TQ + off, (qi + 1) * TQ)
                            stp = psS.tile([TK, TQ], f32, tag="st")
                            nc.tensor.matmul(
                                stp[:, off:],
                                kT[h][:, ki * TK:(ki + 1) * TK],
                                qT[h][:, qsl],
                                start=True, stop=True,
                            )
                            pex = pexp.tile([TK, TQ], bf16, tag="pex")
                            nc.scalar.activation(pex[:, off:], stp[:, off:],
                                                 EXP)
                            if dg >= 0:
                                # zero the invalid triangle of exp in place
                                # on the Pool engine
                                base = off - (dg * TK)  # 0 or -128 (dg=3)
                                blk = 2 * TK if dg == 3 else TK
                                nc.gpsimd.affine_select(
                                    out=pex[:, off:off + blk],
                                    in_=pex[:, off:off + blk],
                                    compare_op=mybir.AluOpType.is_ge,
                                    fill=0.0,
                                    base=base,
                                    pattern=[[1, blk]],
                                    channel_multiplier=-1,
                                )
                            tiles.append((ki, off, pex))
                            # software pipeline: consume the previous tile
                            # while ACT/GPSIMD work on this one
                            if len(tiles) > 1:
                                pki, poff, ppex = tiles.pop(0)
                                nc.tensor.matmul(
                                    outp[:, poff:],
                                    vv[:, pki, h * DH:(h + 1) * DH],
                                    ppex[:, poff:],
                                    start=(pki == 0), stop=False,
                                    skip_group_check=True,
                                )
                                nc.tensor.matmul(
                                    sump[:, poff:],
                                    onesb_sb[:],
                                    ppex[:, poff:],
                                    start=(pki == 0), stop=False,
                                    skip_group_check=True,
                                )
                                pop_y(2)
                        pki, poff, ppex = tiles.pop(0)
                        nc.tensor.matmul(
                            outp[:, poff:],
                            vv[:, pki, h * DH:(h + 1) * DH],
                            ppex[:, poff:],
                            start=(pki == 0), stop=True,
                            skip_group_check=True,
                        )
                        nc.tensor.matmul(
                            sump[:, poff:],
                            onesb_sb[:],
                            ppex[:, poff:],
                            start=(pki == 0), stop=True,
                            skip_group_check=True,
                        )
                        pop_y(2)

                        def norm(h=h, qi=qi, outp=outp, sump=sump, b=b,
                                 ot=otn_tiles):
                            ssb = sax.tile([1, TQ], f32r, tag="ssb", bufs=2,
                                           name="ssb")
                            nc.scalar.copy(ssb[:], sump[:])
                            rbc = psR.tile([DH, TQ], f32, tag="bc",
                                           name="rbc")
                            nc.tensor.matmul(rbc[:], onesrow_sb[:], ssb[:],
                                             start=True, stop=True)
                            rcp = sax.tile([DH, TQ], f32, tag="rcp", bufs=2,
                                           name="rcp")
                            nc.vector.reciprocal(rcp[:], rbc[:])
                            otn = otnp.tile([DH, TQ], bf16, tag="otn",
                                            name="otn")
                            nc.vector.tensor_mul(otn[:], outp[:], rcp[:])
                            ot[(h, qi)] = otn
                            if h != HPC - 1:
                                return

                            def make_ytile(tc2, nch, qi=qi, b=b, ot=ot):
                                def emit():
                                    tq0 = qi * TQ + tc2 * TK
                                    yp = psY.tile([TK, 512], f32,
                                                  tag="y", name="yp")
                                    for hh in range(HPC):
                                        nc.tensor.matmul(
                                            yp[:],
                                            ot[(hh, qi)][
                                                :, tc2 * TK:(tc2 + 1) * TK],
                                            wo_sb[:, hh,
                                                  nch * 512:(nch + 1) * 512],
                                            start=(hh == 0),
                                            stop=(hh == HPC - 1),
                                        )
                                    ysb = pexp.tile([TK, 512], f32,
                                                    tag="ysb", bufs=3,
                                                    name="ysb")
                                    if nch % 2 == 0:
                                        nc.scalar.copy(ysb[:], yp[:])
                                    else:
                                        nc.vector.tensor_copy(ysb[:], yp[:])
                                    nc.sync.dma_start(
                                        y_r[b, tq0:tq0 + TK,
                                            nch * 512:(nch + 1) * 512],
                                        ysb[:])
                                return emit

                            for tc2 in range(TQ // TK):
                                for nch in range(d_sz // 512):
                                    pending_y.append(make_ytile(tc2, nch))

                        pending_norm.append(norm)
                        if len(pending_norm) > 1:
                            pending_norm.pop(0)()
                # drain the last group's norm while its sums are hot; its
                # y-tiles thread into the next batch's projection stream
                pop_norm()
            for fn in pending_norm:
                fn()
            pop_y(len(pending_y))
    if legalize:
        _legalize_waits(nc, mybir)
    return nc


_NC_CACHE = {}
LAST_RESULT = None


def _get_nc(b_sz, t_sz, d_sz):
    key = (b_sz, t_sz, d_sz)
    if key not in _NC_CACHE:
        _NC_CACHE[key] = _build_nc(b_sz, t_sz, d_sz)
    return _NC_CACHE[key]


def kernel(x, w_q, w_k, w_v, w_o):
    import ml_dtypes
    from concourse.bass_utils import run_bass_kernel_spmd

    bf = ml_dtypes.bfloat16
    b_sz, t_sz, d_sz = x.shape
    scale = np.float32(1.0 / np.sqrt(DH))

    xT = np.ascontiguousarray(
        np.asarray(x, np.float32).transpose(0, 2, 1)).astype(bf)
    w_q = np.asarray(w_q, np.float32)
    w_k = np.asarray(w_k, np.float32)
    w_v = np.asarray(w_v, np.float32)
    w_o = np.asarray(w_o, np.float32)
    cosT, sinT = _rope_tables(t_sz, DH, THETA)

    in_maps = []
    for c in range(NCORES):
        cs = slice(c * HPC * DH, (c + 1) * HPC * DH)
        in_maps.append({
            "xT": xT,
            "wq": np.ascontiguousarray(w_q[:, cs] * scale).astype(bf),
            "wk": np.ascontiguousarray(w_k[:, cs]).astype(bf),
            "wv": np.ascontiguousarray(w_v[:, cs]).astype(bf),
            "wo": np.ascontiguousarray(w_o[cs, :]).astype(bf),
            "cos": cosT,
            "sin": sinT,
            "one": np.ones((128, 128), np.float32),
            "oneb": np.ones((128, 128), bf),
        })

    nc = _get_nc(b_sz, t_sz, d_sz)
    res = run_bass_kernel_spmd(nc, in_maps, core_ids=list(range(NCORES)))
    global LAST_RESULT
    LAST_RESULT = res

    out = res.results[0]["y"].astype(np.float32, copy=True)
    for c in range(1, NCORES):
        out += res.results[c]["y"]
    return out
